# revision 1
# baseline (speedup 1.0000x reference)
"""Two-layer GATv2 (DGL GATv2Conv x2 + projection) on 8 Trainium2 NeuronCores.

Sharding: nodes partitioned across 8 cores (1250 each); edges assigned to the
owner of dst; weights replicated; src features exchanged via AllGather of the
per-layer gather table (bf16).

Math: lrelu(z) = 0.6 z + 0.4 |z| (slope 0.2), so the attention logit
e = sum_d a_d lrelu(z_d) = 0.6(as_u + ad_v) + 0.4 sum_d a_d |z_d| with
as = x @ (W_s @ a), ad = x @ (W_d @ a) carried as extra table columns.
Softmax is unnormalized: numerator and denominator accumulate in the same PSUM
window via matmuls with an exp-scaled one-hot scatter matrix; division happens
per 128-node window in the epilogue.

Edge phase: dma_gather fetches up to 1024 edge rows per instruction (src rows
from the allgathered table, dst rows from the local fd table); all elementwise,
activation and reduce work is batched over whole gather blocks.

conv1 table row (640 cols): [fs_h0(256) | 1 | fs_h1(256) | 1 | a0 a1 | 0pad]
  -> per-head agg matmul rhs [fs_h | 1] is contiguous (fused denominator).
conv2 table row (1152 cols): [fs_h0(512) | fs_h1(512) | a0 a1 | 0pad]
  -> denominators via one matmul per chunk with rhs [1 | ex1/ex0].
"""
import numpy as np

N, E = 10000, 160000
IN, HID, OUT, H = 1024, 512, 512, 2
D1 = HID // H
D2 = HID
NCORES = 8
NLOC = N // NCORES
WIN = 128
NW = (NLOC + WIN - 1) // WIN
KBLK = 8

TW1 = 640
TW2 = 1152

_CACHE = {}


def _bf16(x):
    import ml_dtypes
    return np.asarray(x, dtype=np.float32).astype(ml_dtypes.bfloat16)


def _pack_idx16(flat):
    n = len(flat)
    a = np.zeros((16, n // 16), np.int16)
    a[np.arange(n) % 16, np.arange(n) // 16] = flat
    return np.tile(a, (8, 1))


def _host_prep(x, src, dst, W1s, b1s, W1d, b1d, attn1, W1r, b1r,
               W2s, b2s, W2d, b2d, attn2, W2r, b2r, Wp, bp):
    src = np.asarray(src).astype(np.int64)
    dst = np.asarray(dst).astype(np.int64)
    x = np.asarray(x, dtype=np.float32)

    core_of = dst // NLOC
    wloc = (dst % NLOC) // WIN
    e_lists = [[np.nonzero((core_of == m) & (wloc == w))[0] for w in range(NW)]
               for m in range(NCORES)]
    Mw = [max(1, max((len(e_lists[m][w]) + 127) // 128 for m in range(NCORES)))
          for w in range(NW)]
    n_chunks = int(sum(Mw))

    src_idx = np.zeros((NCORES, n_chunks * 128), np.int64)
    dst_idx = np.zeros((NCORES, n_chunks * 128), np.int64)
    oh_ev = np.zeros((NCORES, n_chunks, 128, 128), np.float32)
    for m in range(NCORES):
        ci = 0
        for w in range(NW):
            el = e_lists[m][w]
            ne = len(el)
            npad = Mw[w] * 128
            s_pad = np.zeros(npad, np.int64)
            d_pad = np.zeros(npad, np.int64)
            v_pad = np.zeros(npad, np.int64)
            valid = np.zeros(npad, bool)
            s_pad[:ne] = src[el]
            d_pad[:ne] = dst[el] - m * NLOC
            v_pad[:ne] = dst[el] - m * NLOC - w * WIN
            valid[:ne] = True
            src_idx[m, ci*128:(ci+Mw[w])*128] = s_pad
            dst_idx[m, ci*128:(ci+Mw[w])*128] = d_pad
            for k in range(Mw[w]):
                sl = slice(k * 128, (k + 1) * 128)
                vv, va = v_pad[sl], valid[sl]
                rows = np.nonzero(va)[0]
                oh_ev[m, ci + k, rows, vv[va]] = 1.0
            ci += Mw[w]

    def mk_alpha(W, b, attn, d):
        ac = np.stack([W[:, h*d:(h+1)*d] @ attn[h] for h in range(H)], axis=1) * 0.6
        ab = np.array([0.6 * attn[h] @ b[h*d:(h+1)*d] for h in range(H)], np.float32)
        return ac.astype(np.float32), ab

    attn1 = np.asarray(attn1, np.float32); attn2 = np.asarray(attn2, np.float32)
    W1s = np.asarray(W1s, np.float32); W1d = np.asarray(W1d, np.float32)
    W2s = np.asarray(W2s, np.float32); W2d = np.asarray(W2d, np.float32)
    b1s = np.asarray(b1s, np.float32); b1d = np.asarray(b1d, np.float32)
    b2s = np.asarray(b2s, np.float32); b2d = np.asarray(b2d, np.float32)
    a1s, a1s_b = mk_alpha(W1s, b1s, attn1, D1)
    a1d, a1d_b = mk_alpha(W1d, b1d, attn1, D1)
    a2s, a2s_b = mk_alpha(W2s, b2s, attn2, D2)
    a2d, a2d_b = mk_alpha(W2d, b2d, attn2, D2)

    # conv1 T block: [fs0 | ones | fs1 | ones | a0 a1 | pad]
    def blk1(W, alpha):
        B = np.zeros((IN, TW1), np.float32)
        B[:, 0:256] = W[:, 0:256]
        B[:, 257:513] = W[:, 256:512]
        B[:, 514:516] = alpha
        return B

    def brow1(b, ab, with_ones):
        r = np.zeros(TW1, np.float32)
        r[0:256] = b[0:256]; r[257:513] = b[256:512]; r[514:516] = ab
        if with_ones:
            r[256] = 1.0; r[513] = 1.0
        return r

    W1cat = np.concatenate([blk1(W1s, a1s), blk1(W1d, a1d), np.asarray(W1r, np.float32)], axis=1)
    b1cat = np.zeros((128, W1cat.shape[1]), np.float32)
    b1cat[0, 0:TW1] = brow1(b1s, a1s_b, True)
    b1cat[0, TW1:2*TW1] = brow1(b1d, a1d_b, False)
    b1cat[0, 2*TW1:] = np.asarray(b1r, np.float32)

    # conv2 T block: [fs0 | fs1 | a0 a1 | pad]
    def blk2(W, alpha):
        B = np.zeros((HID, TW2), np.float32)
        B[:, 0:1024] = W
        B[:, 1024:1026] = alpha
        return B

    W2cat = np.concatenate([blk2(W2s, a2s), blk2(W2d, a2d), np.asarray(W2r, np.float32)], axis=1)
    b2cat = np.zeros((128, W2cat.shape[1]), np.float32)
    b2cat[0, 0:1024] = b2s; b2cat[0, 1024:1026] = a2s_b
    b2cat[0, TW2:TW2+1024] = b2d; b2cat[0, TW2+1024:TW2+1026] = a2d_b
    b2cat[0, 2*TW2:] = np.asarray(b2r, np.float32)

    bpcat = np.zeros((128, OUT), np.float32)
    bpcat[0, :] = np.asarray(bp, np.float32)
    has_bias = bool(max(float(np.abs(np.asarray(b, np.float32)).max()) for b in
                        (b1s, b1d, b1r, b2s, b2d, b2r, bp)) > 0)

    attn1_b = np.tile((0.4 * attn1.reshape(1, -1)), (128, 1))
    attn2_b = np.tile((0.4 * attn2.reshape(1, -1)), (128, 1))
    ident = np.eye(128, dtype=np.float32)
    ebias = np.zeros((128, 128), np.float32); ebias[0, :] = 1.0

    shared = {
        "w1cat": _bf16(W1cat), "b1cat": _bf16(b1cat),
        "w2cat": _bf16(W2cat), "b2cat": _bf16(b2cat),
        "wp": _bf16(np.asarray(Wp, np.float32)), "bpcat": _bf16(bpcat),
        "attn1b": _bf16(attn1_b), "attn2b": _bf16(attn2_b),
        "ident": _bf16(ident), "ebias": _bf16(ebias),
    }
    in_maps = []
    for m in range(NCORES):
        xm = x[m*NLOC:(m+1)*NLOC]
        xT = np.zeros((IN, 1280), np.float32)
        xT[:, :NLOC] = xm.T
        im = dict(shared)
        im["xt"] = _bf16(xT)
        im["sidx"] = _pack_idx16(src_idx[m])
        im["didx"] = _pack_idx16(dst_idx[m])
        im["ohev"] = _bf16(oh_ev[m])
        in_maps.append(im)
    return in_maps, Mw, n_chunks, has_bias


def _blocks(mw):
    out, c = [], 0
    while c < mw:
        k = min(KBLK, mw - c)
        out.append((c, k))
        c += k
    return out


def _build_program(Mw, n_chunks, has_bias=False, repeat=1):
    import sys
    if "/opt/trn_rl_repo" not in sys.path:
        sys.path.insert(0, "/opt/trn_rl_repo")
    import concourse.bass as bass
    import concourse.bacc as bacc
    import concourse.mybir as mybir
    import concourse.tile as tile

    dt = mybir.dt
    AF = mybir.ActivationFunctionType
    AL = mybir.AluOpType

    nc = bacc.Bacc("TRN2", target_bir_lowering=False, debug=False,
                   num_devices=NCORES)

    W1W = 2 * TW1 + 512    # 1792
    W2W = 2 * TW2 + 1024   # 3328
    RG = [list(range(NCORES))]

    xt_d = nc.dram_tensor("xt", [IN, 1280], dt.bfloat16, kind="ExternalInput")
    w1_d = nc.dram_tensor("w1cat", [IN, W1W], dt.bfloat16, kind="ExternalInput")
    b1_d = nc.dram_tensor("b1cat", [128, W1W], dt.bfloat16, kind="ExternalInput")
    w2_d = nc.dram_tensor("w2cat", [HID, W2W], dt.bfloat16, kind="ExternalInput")
    b2_d = nc.dram_tensor("b2cat", [128, W2W], dt.bfloat16, kind="ExternalInput")
    wp_d = nc.dram_tensor("wp", [HID, OUT], dt.bfloat16, kind="ExternalInput")
    bp_d = nc.dram_tensor("bpcat", [128, OUT], dt.bfloat16, kind="ExternalInput")
    at1_d = nc.dram_tensor("attn1b", [128, 512], dt.bfloat16, kind="ExternalInput")
    at2_d = nc.dram_tensor("attn2b", [128, 1024], dt.bfloat16, kind="ExternalInput")
    id_d = nc.dram_tensor("ident", [128, 128], dt.bfloat16, kind="ExternalInput")
    eb_d = nc.dram_tensor("ebias", [128, 128], dt.bfloat16, kind="ExternalInput")
    si_d = nc.dram_tensor("sidx", [128, n_chunks * 8], dt.int16, kind="ExternalInput")
    di_d = nc.dram_tensor("didx", [128, n_chunks * 8], dt.int16, kind="ExternalInput")
    ohev_d = nc.dram_tensor("ohev", [n_chunks, 128, 128], dt.bfloat16, kind="ExternalInput")

    t1_own = nc.dram_tensor("t1_own", [NLOC, TW1], dt.bfloat16, kind="Internal")
    t1_full = nc.dram_tensor("t1_full", [N, TW1], dt.bfloat16, kind="Internal",
                             addr_space="Shared")
    fd1_dram = nc.dram_tensor("fd1_dram", [NLOC, TW1], dt.bfloat16, kind="Internal")
    t2_own = nc.dram_tensor("t2_own", [NLOC, TW2], dt.bfloat16, kind="Internal")
    t2_full = nc.dram_tensor("t2_full", [N, TW2], dt.bfloat16, kind="Internal",
                             addr_space="Shared")
    fd2_dram = nc.dram_tensor("fd2_dram", [NLOC, TW2], dt.bfloat16, kind="Internal")
    out_d = nc.dram_tensor("out", [NLOC, OUT], dt.float32, kind="ExternalOutput")

    def mm_cols(ncols):
        splits, c = [], 0
        while c < ncols:
            n_ = min(512, ncols - c)
            splits.append((c, c + n_))
            c += n_
        return splits

    def node_matmul(ps, lhs_of_k, nk, w_sb, ww, b_sb, ones_cols, ebias):
        """Accumulate sum_k lhsT_k.T @ W_k into ps[:, 0:ww] (+ bias row)."""
        spans = mm_cols(ww)
        for k in range(nk):
            lhs = lhs_of_k(k)
            for si, (c0, c1) in enumerate(spans):
                last = (k == nk - 1)
                need_bias = has_bias or any(c0 <= oc < c1 for oc in ones_cols)
                nc.tensor.matmul(ps[:, c0:c1], lhsT=lhs, rhs=w_sb[:, k*ww+c0:k*ww+c1],
                                 start=(k == 0), stop=(last and not need_bias))
        for (c0, c1) in spans:
            need_bias = has_bias or any(c0 <= oc < c1 for oc in ones_cols)
            if need_bias:
                nc.tensor.matmul(ps[:, c0:c1], lhsT=ebias[:, :], rhs=b_sb[:, c0:c1],
                                 start=False, stop=True)

    with tile.TileContext(nc) as tc:
        with tc.tile_pool(name="cst", bufs=1) as cst, \
             tc.tile_pool(name="res", bufs=1) as resid:

            ident = cst.tile([128, 128], dt.bfloat16)
            nc.sync.dma_start(out=ident[:, :], in_=id_d[:, :])
            ebias = cst.tile([128, 128], dt.bfloat16)
            nc.sync.dma_start(out=ebias[:, :], in_=eb_d[:, :])
            at1 = cst.tile([128, 512], dt.bfloat16)
            nc.sync.dma_start(out=at1[:, :], in_=at1_d[:, :])
            at2 = cst.tile([128, 1024], dt.bfloat16)
            nc.sync.dma_start(out=at2[:, :], in_=at2_d[:, :])
            sidx = cst.tile([128, n_chunks * 8], dt.int16)
            nc.sync.dma_start(out=sidx[:, :], in_=si_d[:, :])
            didx = cst.tile([128, n_chunks * 8], dt.int16)
            nc.sync.dma_start(out=didx[:, :], in_=di_d[:, :])

            rs1 = resid.tile([128, NW * 512], dt.bfloat16)
            h1 = resid.tile([128, NW * 512], dt.bfloat16)
            rs2 = resid.tile([128, NW * 1024], dt.bfloat16)

            for rep in range(repeat):
                # ================= conv1 node matmuls =================
                with tc.tile_pool(name="n1", bufs=2) as n1p, \
                     tc.tile_pool(name="n1w", bufs=1) as n1w, \
                     tc.tile_pool(name="n1ps", bufs=1, space="PSUM") as n1ps:
                    xt_sb = n1w.tile([128, 8 * 1280], dt.bfloat16)
                    for k in range(8):
                        nc.sync.dma_start(out=xt_sb[:, k*1280:(k+1)*1280],
                                          in_=xt_d[k*128:(k+1)*128, :])
                    w1 = n1w.tile([128, 8 * W1W], dt.bfloat16)
                    for k in range(8):
                        nc.sync.dma_start(out=w1[:, k*W1W:(k+1)*W1W],
                                          in_=w1_d[k*128:(k+1)*128, :])
                    b1 = n1w.tile([128, W1W], dt.bfloat16)
                    nc.sync.dma_start(out=b1[:, :], in_=b1_d[:, :])
                    for nt in range(NW):
                        ps = n1ps.tile([128, W1W], dt.float32, space="PSUM")
                        node_matmul(ps,
                                    lambda k: xt_sb[:, k*1280 + nt*128: k*1280 + (nt+1)*128],
                                    8, w1, W1W, b1, (256, 513), ebias)
                        t1t = n1p.tile([128, TW1], dt.bfloat16, tag="t1t")
                        nc.vector.tensor_copy(out=t1t[:, :], in_=ps[:, 0:TW1])
                        fdt = n1p.tile([128, TW1], dt.bfloat16, tag="fdt")
                        nc.vector.tensor_copy(out=fdt[:, :], in_=ps[:, TW1:2*TW1])
                        nc.vector.tensor_copy(out=rs1[:, nt*512:(nt+1)*512],
                                              in_=ps[:, 2*TW1:W1W])
                        rows = min(128, NLOC - nt * 128)
                        nc.sync.dma_start(out=t1_own[nt*128:nt*128+rows, :],
                                          in_=t1t[:rows, :])
                        nc.sync.dma_start(out=fd1_dram[nt*128:nt*128+rows, :],
                                          in_=fdt[:rows, :])

                nc.gpsimd.collective_compute(
                    "AllGather", mybir.AluOpType.bypass, replica_groups=RG,
                    ins=[t1_own[:, :]], outs=[t1_full[:, :]])

                # ================= conv1 edge phase =================
                with tc.tile_pool(name="e1a", bufs=2) as e1a, \
                     tc.tile_pool(name="e1b", bufs=1) as e1b, \
                     tc.tile_pool(name="e1agg", bufs=2, space="PSUM") as e1agg:
                    cbase = 0
                    for w in range(NW):
                        mw = Mw[w]
                        agg0 = e1agg.tile([128, 257], dt.float32, space="PSUM", tag="agg0")
                        agg1 = e1agg.tile([128, 257], dt.float32, space="PSUM", tag="agg1")
                        for bi, (c0, kb) in enumerate(_blocks(mw)):
                            ci = cbase + c0
                            first = (bi == 0)
                            last = (c0 + kb == mw)
                            gfs = e1a.tile([128, KBLK * TW1], dt.bfloat16, tag="gfs")
                            nc.gpsimd.dma_gather(
                                out_ap=gfs[:, 0:kb*TW1].rearrange("p (k t) -> p k t", t=TW1),
                                in_ap=t1_full[:, :], idxs_ap=sidx[:, ci*8:(ci+kb)*8],
                                num_idxs=kb*128, num_idxs_reg=kb*128, elem_size=TW1)
                            gfd = e1b.tile([128, KBLK * TW1], dt.bfloat16, tag="gfd")
                            nc.gpsimd.dma_gather(
                                out_ap=gfd[:, 0:kb*TW1].rearrange("p (k t) -> p k t", t=TW1),
                                in_ap=fd1_dram[:, :], idxs_ap=didx[:, ci*8:(ci+kb)*8],
                                num_idxs=kb*128, num_idxs_reg=kb*128, elem_size=TW1)
                            oh = e1a.tile([128, KBLK * 128], dt.bfloat16, tag="oh")
                            nc.sync.dma_start(
                                out=oh[:, 0:kb*128].rearrange("p (c o) -> p c o", o=128),
                                in_=ohev_d[ci:ci+kb, :, :].rearrange("c p o -> p c o"))
                            z = e1b.tile([128, KBLK * TW1], dt.float32, tag="z")
                            nc.vector.tensor_tensor(out=z[:, 0:kb*TW1], in0=gfs[:, 0:kb*TW1],
                                                    in1=gfd[:, 0:kb*TW1], op=AL.add)
                            zv = z[:, 0:kb*TW1].rearrange("p (k t) -> p k t", t=TW1)
                            ab = e1b.tile([128, KBLK * 512], dt.bfloat16, tag="ab")
                            abv = ab[:, 0:kb*512].rearrange("p (k h d) -> p k h d", h=2, d=256)
                            nc.scalar.activation(
                                out=abv,
                                in_=z[:, 0:kb*TW1].rearrange("p (k t) -> p k t", t=TW1)
                                    [:, :, 0:514].rearrange("p k (h d) -> p k h d", d=257)
                                    [:, :, :, 0:256],
                                func=AF.Abs)
                            nc.vector.tensor_tensor(
                                out=ab[:, 0:kb*512].rearrange("p (k d) -> p k d", d=512),
                                in0=ab[:, 0:kb*512].rearrange("p (k d) -> p k d", d=512),
                                in1=at1[:, :].rearrange("p (u d) -> p u d", u=1)
                                    .broadcast_to([128, kb, 512]),
                                op=AL.mult)
                            eab = e1b.tile([128, 2 * KBLK], dt.float32, tag="eab")
                            nc.vector.tensor_reduce(
                                out=eab[:, 0:2*kb],
                                in_=ab[:, 0:kb*512].rearrange("p (g d) -> p g d", d=256),
                                axis=mybir.AxisListType.X, op=AL.add)
                            e2 = e1b.tile([128, 2 * KBLK], dt.float32, tag="e2")
                            nc.vector.tensor_tensor(
                                out=e2[:, 0:2*kb].rearrange("p (k h) -> p k h", h=2),
                                in0=eab[:, 0:2*kb].rearrange("p (k h) -> p k h", h=2),
                                in1=zv[:, :, 514:516], op=AL.add)
                            ex = e1b.tile([128, 2 * KBLK], dt.float32, tag="ex")
                            nc.scalar.activation(out=ex[:, 0:2*kb], in_=e2[:, 0:2*kb],
                                                 func=AF.Exp)
                            sS = e1a.tile([128, 2 * KBLK * 128], dt.bfloat16, tag="sS")
                            for h in range(2):
                                nc.vector.tensor_tensor(
                                    out=sS[:, h*KBLK*128:h*KBLK*128+kb*128]
                                        .rearrange("p (c o) -> p c o", o=128),
                                    in0=oh[:, 0:kb*128].rearrange("p (c o) -> p c o", o=128),
                                    in1=ex[:, h:2*kb:2].rearrange("p (c u) -> p c u", u=1)
                                        .broadcast_to([128, kb, 128]),
                                    op=AL.mult)
                            for c in range(kb):
                                st = first and (c == 0)
                                sp = last and (c == kb - 1)
                                nc.tensor.matmul(
                                    agg0[:, :], lhsT=sS[:, c*128:(c+1)*128],
                                    rhs=gfs[:, c*TW1:c*TW1+257], start=st, stop=sp)
                                nc.tensor.matmul(
                                    agg1[:, :], lhsT=sS[:, KBLK*128+c*128:KBLK*128+(c+1)*128],
                                    rhs=gfs[:, c*TW1+257:c*TW1+514], start=st, stop=sp)
                        # ---- window epilogue ----
                        dsb = e1b.tile([128, 2], dt.float32, tag="dsb")
                        nc.vector.tensor_copy(out=dsb[:, 0:1], in_=agg0[:, 256:257])
                        nc.vector.tensor_copy(out=dsb[:, 1:2], in_=agg1[:, 256:257])
                        nc.vector.tensor_scalar_add(dsb[:, :], dsb[:, :], 1e-20)
                        rcp = e1b.tile([128, 2], dt.float32, tag="rcp")
                        nc.vector.reciprocal(rcp[:, :], dsb[:, :])
                        hw = h1[:, w*512:(w+1)*512]
                        nc.vector.tensor_scalar_mul(hw[:, 0:256], agg0[:, 0:256], rcp[:, 0:1])
                        nc.vector.tensor_scalar_mul(hw[:, 256:512], agg1[:, 0:256], rcp[:, 1:2])
                        nc.vector.tensor_tensor(out=hw[:, :], in0=hw[:, :],
                                                in1=rs1[:, w*512:(w+1)*512], op=AL.add)
                        nc.vector.tensor_scalar_max(hw[:, :], hw[:, :], 0.0)
                        cbase += mw

                # ================= conv2 node matmuls =================
                with tc.tile_pool(name="n2", bufs=2) as n2p, \
                     tc.tile_pool(name="n2w", bufs=1) as n2w, \
                     tc.tile_pool(name="n2ps", bufs=1, space="PSUM") as n2ps:
                    w2 = n2w.tile([128, 4 * W2W], dt.bfloat16)
                    for k in range(4):
                        nc.sync.dma_start(out=w2[:, k*W2W:(k+1)*W2W],
                                          in_=w2_d[k*128:(k+1)*128, :])
                    b2 = n2w.tile([128, W2W], dt.bfloat16)
                    nc.sync.dma_start(out=b2[:, :], in_=b2_d[:, :])
                    HALF = 1792   # bank-aligned split of 3328 (1792 + 1536)
                    for nt in range(NW):
                        tp = n2ps.tile([128, 128], dt.bfloat16, space="PSUM", tag="tp")
                        hT = n2p.tile([128, 512], dt.bfloat16, tag="hT")
                        for j in range(4):
                            nc.tensor.transpose(out=tp[:, :],
                                                in_=h1[:, nt*512+j*128:nt*512+(j+1)*128],
                                                identity=ident[:, :])
                            nc.vector.tensor_copy(out=hT[:, j*128:(j+1)*128], in_=tp[:, :])
                        t2t = n2p.tile([128, TW2], dt.bfloat16, tag="t2t")
                        fdt = n2p.tile([128, TW2], dt.bfloat16, tag="fdt2")
                        ps = n2ps.tile([128, HALF], dt.float32, space="PSUM")
                        for hf, (base, hw_) in enumerate(((0, HALF), (HALF, W2W - HALF))):
                            spans = mm_cols(hw_)
                            for k in range(4):
                                for (c0, c1) in spans:
                                    nc.tensor.matmul(
                                        ps[:, c0:c1], lhsT=hT[:, k*128:(k+1)*128],
                                        rhs=w2[:, k*W2W+base+c0:k*W2W+base+c1],
                                        start=(k == 0),
                                        stop=(k == 3 and not has_bias))
                            if has_bias:
                                for (c0, c1) in spans:
                                    nc.tensor.matmul(ps[:, c0:c1], lhsT=ebias[:, :],
                                                     rhs=b2[:, base+c0:base+c1],
                                                     start=False, stop=True)
                            seg = [(0, TW2, "t2"), (TW2, 2*TW2, "fd"), (2*TW2, W2W, "rs")]
                            for (s0, s1, kind) in seg:
                                lo, hi = max(s0, base), min(s1, base + hw_)
                                if lo >= hi:
                                    continue
                                srcv = ps[:, lo-base:hi-base]
                                if kind == "t2":
                                    nc.vector.tensor_copy(out=t2t[:, lo:hi], in_=srcv)
                                elif kind == "fd":
                                    nc.vector.tensor_copy(out=fdt[:, lo-TW2:hi-TW2], in_=srcv)
                                else:
                                    nc.vector.tensor_copy(
                                        out=rs2[:, nt*1024+lo-2*TW2:nt*1024+hi-2*TW2],
                                        in_=srcv)
                        rows = min(128, NLOC - nt * 128)
                        nc.sync.dma_start(out=t2_own[nt*128:nt*128+rows, :],
                                          in_=t2t[:rows, :])
                        nc.sync.dma_start(out=fd2_dram[nt*128:nt*128+rows, :],
                                          in_=fdt[:rows, :])

                nc.gpsimd.collective_compute(
                    "AllGather", mybir.AluOpType.bypass, replica_groups=RG,
                    ins=[t2_own[:, :]], outs=[t2_full[:, :]])

                # ================= conv2 edge phase + final =================
                with tc.tile_pool(name="e2a", bufs=2) as e2a, \
                     tc.tile_pool(name="e2b", bufs=1) as e2b, \
                     tc.tile_pool(name="e2w", bufs=1) as e2w, \
                     tc.tile_pool(name="e2agg", bufs=2, space="PSUM") as e2agg, \
                     tc.tile_pool(name="e2pf", bufs=1, space="PSUM") as e2pf:
                    wp = e2w.tile([128, 4 * OUT], dt.bfloat16)
                    for k in range(4):
                        nc.sync.dma_start(out=wp[:, k*OUT:(k+1)*OUT],
                                          in_=wp_d[k*128:(k+1)*128, :])
                    bpt = e2w.tile([128, OUT], dt.bfloat16)
                    nc.sync.dma_start(out=bpt[:, :], in_=bp_d[:, :])
                    cbase = 0
                    for w in range(NW):
                        mw = Mw[w]
                        agg0 = e2agg.tile([128, 512], dt.float32, space="PSUM", tag="agg0")
                        agg1 = e2agg.tile([128, 512], dt.float32, space="PSUM", tag="agg1")
                        den = e2agg.tile([128, 2], dt.float32, space="PSUM", tag="den")
                        for bi, (c0, kb) in enumerate(_blocks(mw)):
                            ci = cbase + c0
                            first = (bi == 0)
                            last = (c0 + kb == mw)
                            gfs = e2a.tile([128, KBLK * TW2], dt.bfloat16, tag="gfs")
                            nc.gpsimd.dma_gather(
                                out_ap=gfs[:, 0:kb*TW2].rearrange("p (k t) -> p k t", t=TW2),
                                in_ap=t2_full[:, :], idxs_ap=sidx[:, ci*8:(ci+kb)*8],
                                num_idxs=kb*128, num_idxs_reg=kb*128, elem_size=TW2)
                            gfd = e2b.tile([128, KBLK * TW2], dt.bfloat16, tag="gfd")
                            nc.gpsimd.dma_gather(
                                out_ap=gfd[:, 0:kb*TW2].rearrange("p (k t) -> p k t", t=TW2),
                                in_ap=fd2_dram[:, :], idxs_ap=didx[:, ci*8:(ci+kb)*8],
                                num_idxs=kb*128, num_idxs_reg=kb*128, elem_size=TW2)
                            oh = e2a.tile([128, KBLK * 128], dt.bfloat16, tag="oh")
                            nc.sync.dma_start(
                                out=oh[:, 0:kb*128].rearrange("p (c o) -> p c o", o=128),
                                in_=ohev_d[ci:ci+kb, :, :].rearrange("c p o -> p c o"))
                            z = e2b.tile([128, KBLK * TW2], dt.float32, tag="z")
                            nc.vector.tensor_tensor(out=z[:, 0:kb*TW2], in0=gfs[:, 0:kb*TW2],
                                                    in1=gfd[:, 0:kb*TW2], op=AL.add)
                            zv = z[:, 0:kb*TW2].rearrange("p (k t) -> p k t", t=TW2)
                            ab = e2b.tile([128, KBLK * 1024], dt.bfloat16, tag="ab")
                            nc.scalar.activation(
                                out=ab[:, 0:kb*1024].rearrange("p (k d) -> p k d", d=1024),
                                in_=zv[:, :, 0:1024], func=AF.Abs)
                            nc.vector.tensor_tensor(
                                out=ab[:, 0:kb*1024].rearrange("p (k d) -> p k d", d=1024),
                                in0=ab[:, 0:kb*1024].rearrange("p (k d) -> p k d", d=1024),
                                in1=at2[:, :].rearrange("p (u d) -> p u d", u=1)
                                    .broadcast_to([128, kb, 1024]),
                                op=AL.mult)
                            eab = e2b.tile([128, 2 * KBLK], dt.float32, tag="eab")
                            nc.vector.tensor_reduce(
                                out=eab[:, 0:2*kb],
                                in_=ab[:, 0:kb*1024].rearrange("p (g d) -> p g d", d=512),
                                axis=mybir.AxisListType.X, op=AL.add)
                            e2t = e2b.tile([128, 2 * KBLK], dt.float32, tag="e2t")
                            nc.vector.tensor_tensor(
                                out=e2t[:, 0:2*kb].rearrange("p (k h) -> p k h", h=2),
                                in0=eab[:, 0:2*kb].rearrange("p (k h) -> p k h", h=2),
                                in1=zv[:, :, 1024:1026], op=AL.add)
                            ex = e2b.tile([128, 2 * KBLK], dt.float32, tag="ex")
                            nc.scalar.activation(out=ex[:, 0:2*kb], in_=e2t[:, 0:2*kb],
                                                 func=AF.Exp)
                            # denominator rhs: [1 | ex1/ex0] per chunk
                            dn = e2b.tile([128, 2 * KBLK], dt.bfloat16, tag="dn")
                            nc.vector.memset(dn[:, 0:2*kb], 1.0)
                            rr = e2b.tile([128, KBLK], dt.float32, tag="rr")
                            nc.vector.reciprocal(rr[:, 0:kb], ex[:, 0:2*kb:2])
                            nc.vector.tensor_tensor(out=dn[:, 1:2*kb:2], in0=rr[:, 0:kb],
                                                    in1=ex[:, 1:2*kb:2], op=AL.mult)
                            sS = e2a.tile([128, 2 * KBLK * 128], dt.bfloat16, tag="sS")
                            for h in range(2):
                                nc.vector.tensor_tensor(
                                    out=sS[:, h*KBLK*128:h*KBLK*128+kb*128]
                                        .rearrange("p (c o) -> p c o", o=128),
                                    in0=oh[:, 0:kb*128].rearrange("p (c o) -> p c o", o=128),
                                    in1=ex[:, h:2*kb:2].rearrange("p (c u) -> p c u", u=1)
                                        .broadcast_to([128, kb, 128]),
                                    op=AL.mult)
                            for c in range(kb):
                                st = first and (c == 0)
                                sp = last and (c == kb - 1)
                                nc.tensor.matmul(
                                    agg0[:, :], lhsT=sS[:, c*128:(c+1)*128],
                                    rhs=gfs[:, c*TW2:c*TW2+512], start=st, stop=sp)
                                nc.tensor.matmul(
                                    agg1[:, :], lhsT=sS[:, KBLK*128+c*128:KBLK*128+(c+1)*128],
                                    rhs=gfs[:, c*TW2+512:c*TW2+1024], start=st, stop=sp)
                                nc.tensor.matmul(
                                    den[:, :], lhsT=sS[:, c*128:(c+1)*128],
                                    rhs=dn[:, 2*c:2*c+2], start=st, stop=sp)
                        # ---- epilogue: h2 = sum_h relu(num/den + res) ----
                        dsb = e2b.tile([128, 2], dt.float32, tag="dsb")
                        nc.vector.tensor_scalar_add(dsb[:, :], den[:, :], 1e-20)
                        rcp = e2b.tile([128, 2], dt.float32, tag="rcp")
                        nc.vector.reciprocal(rcp[:, :], dsb[:, :])
                        th0 = e2b.tile([128, 512], dt.bfloat16, tag="th0")
                        th1 = e2b.tile([128, 512], dt.bfloat16, tag="th1")
                        nc.vector.tensor_scalar_mul(th0[:, :], agg0[:, :], rcp[:, 0:1])
                        nc.vector.tensor_scalar_mul(th1[:, :], agg1[:, :], rcp[:, 1:2])
                        nc.vector.tensor_tensor(out=th0[:, :], in0=th0[:, :],
                                                in1=rs2[:, w*1024:w*1024+512], op=AL.add)
                        nc.vector.tensor_tensor(out=th1[:, :], in0=th1[:, :],
                                                in1=rs2[:, w*1024+512:(w+1)*1024], op=AL.add)
                        nc.vector.tensor_scalar_max(th0[:, :], th0[:, :], 0.0)
                        nc.vector.tensor_scalar_max(th1[:, :], th1[:, :], 0.0)
                        h2w = e2b.tile([128, 512], dt.bfloat16, tag="h2w")
                        nc.vector.tensor_tensor(out=h2w[:, :], in0=th0[:, :],
                                                in1=th1[:, :], op=AL.add)
                        # ---- final projection ----
                        pf = e2pf.tile([128, OUT], dt.float32, space="PSUM", tag="pf")
                        h2T = e2b.tile([128, 512], dt.bfloat16, tag="h2T")
                        for j in range(4):
                            tpv = pf[:, j*64:(j+1)*64].bitcast(dt.bfloat16)
                            nc.tensor.transpose(out=tpv, in_=h2w[:, j*128:(j+1)*128],
                                                identity=ident[:, :])
                            nc.vector.tensor_copy(out=h2T[:, j*128:(j+1)*128], in_=tpv)
                        for k in range(4):
                            nc.tensor.matmul(pf[:, :], lhsT=h2T[:, k*128:(k+1)*128],
                                             rhs=wp[:, k*OUT:(k+1)*OUT],
                                             start=(k == 0), stop=(k == 3 and not has_bias))
                        if has_bias:
                            nc.tensor.matmul(pf[:, :], lhsT=ebias[:, :], rhs=bpt[:, :],
                                             start=False, stop=True)
                        of = e2b.tile([128, OUT], dt.float32, tag="of")
                        nc.vector.tensor_copy(out=of[:, :], in_=pf[:, :])
                        rows = min(128, NLOC - w * 128)
                        nc.sync.dma_start(out=out_d[w*128:w*128+rows, :],
                                          in_=of[:rows, :])
                        cbase += mw

    nc.compile()
    return nc


def kernel(**inputs) -> np.ndarray:
    import sys
    if "/opt/trn_rl_repo" not in sys.path:
        sys.path.insert(0, "/opt/trn_rl_repo")
    from concourse.bass_utils import run_bass_kernel_spmd

    in_maps, Mw, n_chunks, has_bias = _host_prep(**inputs)
    key = ("prog", tuple(Mw), has_bias)
    if key not in _CACHE:
        _CACHE[key] = _build_program(Mw, n_chunks, has_bias)
    nc = _CACHE[key]
    res = run_bass_kernel_spmd(nc, in_maps, core_ids=list(range(NCORES)))
    return np.concatenate([res.results[m]["out"] for m in range(NCORES)], axis=0)



# revision 14
# speedup vs baseline: 2683.7706x; 2683.7706x over previous
"""Two-layer GATv2 (DGL GATv2Conv x2 + projection) on 8 Trainium2 NeuronCores.

Sharding: nodes partitioned across 8 cores (1250 each); edges assigned to the
owner of dst; weights replicated; src features exchanged via AllGather of the
per-layer gather table (bf16).

V3 design:
- Attention vector folded into the tables: column j of the fs/fd blocks stores
  0.4*a_d*fs_d with columns sign-sorted per head (positives first), so the
  logit 0.4*sum_d a_d|z_d| = R(+seg) - R(-seg) comes from abs-reduces
  (DVE tensor_reduce(apply_absolute_value) / ACT Abs+accum_out, split across
  both engines) with no separate abs or multiply pass. The 0.6*(as_u+ad_v)
  part rides along as two extra table columns.
- Aggregation uses sum_e exp_e * Z where Z = fs'_u + fd'_v: equals
  sum exp*fs' + den*fd'_v, so the window epilogue subtracts fd'_v after
  dividing by den, then unscales by 1/(0.4 a_d); column permutations are
  folded into downstream weights host-side (incl. per-head-permuted Wp with
  the head-sum pushed through the final relu, which is identity here).
- Per-window pipeline: conv1 edge(w) -> conv2 node matmuls(w) -> chunked
  AllGather piece, so the conv2 AllGather streams behind conv1 edge compute.
  Gather tables are laid out [piece, core, row] to make chunked AG output
  directly gatherable.
"""
import numpy as np

N, E = 10000, 160000
IN, HID, OUT, H = 1024, 512, 512, 2
D1 = HID // H   # 256
D2 = HID        # 512
NCORES = 8
NLOC = N // NCORES
WIN = 128
NW = (NLOC + WIN - 1) // WIN
KBLK = 8
AGW = 10                     # windows per AllGather piece
NPC = NW // AGW              # AG pieces per layer
N2 = NPC * NCORES * WIN * AGW   # rows in gather tables (10240)

TW1 = 640    # [h0'(256) | h1'(256) | as0 as1 | pad]; 1280B (%256==0)
TW2 = 1152   # [h0'(512) | h1'(512) | as0 as1 | pad]; 2304B

_CACHE = {}


def _bf16(x):
    import ml_dtypes
    return np.asarray(x, dtype=np.float32).astype(ml_dtypes.bfloat16)


def _pack_idx16(flat):
    n = len(flat)
    a = np.zeros((16, n // 16), np.int16)
    a[np.arange(n) % 16, np.arange(n) // 16] = flat
    return np.tile(a, (8, 1))


def _row_of(u):
    """Global node id -> row in the [piece, core, AGW*WIN] gather table."""
    m = u // NLOC
    loc = u % NLOC
    w = loc // WIN
    r = loc % WIN
    return ((w // AGW) * (NCORES * WIN * AGW) + m * (WIN * AGW)
            + (w % AGW) * WIN + r)


def _fold_tables(Ws, bs, Wd, bd, attn, Wr, br, d, TW):
    """a-folded sign-sorted fs/fd tables + permuted residual + metadata."""
    Hh = attn.shape[0]
    perms, nps, s_perm = [], [], []
    for h in range(Hh):
        s = 0.4 * attn[h]
        pos = s >= 0
        perm = np.concatenate([np.nonzero(pos)[0], np.nonzero(~pos)[0]])
        perms.append(perm)
        np_h = int(pos.sum())
        assert 0 < np_h < d, "empty sign segment: add memset fallback"
        nps.append(np_h)
        sp = s[perm]
        assert np.abs(sp).min() > 1e-12
        s_perm.append(sp)

    def tab(W, b):
        T = np.zeros((W.shape[0], TW), np.float32)
        trow = np.zeros((TW,), np.float32)
        for h in range(Hh):
            T[:, h*d:(h+1)*d] = W[:, h*d:(h+1)*d][:, perms[h]] * s_perm[h][None, :]
            trow[h*d:(h+1)*d] = b[h*d:(h+1)*d][perms[h]] * s_perm[h]
            T[:, Hh*d + h] = 0.6 * (W[:, h*d:(h+1)*d] @ attn[h])
            trow[Hh*d + h] = 0.6 * float(attn[h] @ b[h*d:(h+1)*d])
        return T, trow

    Ts, brow_s = tab(Ws, bs)
    Td, brow_d = tab(Wd, bd)
    Wr_p = np.zeros_like(Wr, dtype=np.float32)
    br_p = np.zeros(Hh * d, np.float32)
    for h in range(Hh):
        Wr_p[:, h*d:(h+1)*d] = Wr[:, h*d:(h+1)*d][:, perms[h]]
        br_p[h*d:(h+1)*d] = br[h*d:(h+1)*d][perms[h]]
    inv_s = np.concatenate([1.0 / s_perm[h] for h in range(Hh)]).astype(np.float32)
    rowperm = np.concatenate([perms[h] + h * d for h in range(Hh)])
    return Ts, brow_s, Td, brow_d, Wr_p, br_p, inv_s, rowperm, tuple(nps)


def _host_prep(x, src, dst, W1s, b1s, W1d, b1d, attn1, W1r, b1r,
               W2s, b2s, W2d, b2d, attn2, W2r, b2r, Wp, bp):
    src = np.asarray(src).astype(np.int64)
    dst = np.asarray(dst).astype(np.int64)
    x = np.asarray(x, dtype=np.float32)
    deg = np.bincount(dst, minlength=N)
    assert deg.min() >= 1, "zero in-degree node: epilogue fd-subtraction invalid"

    core_of = dst // NLOC
    wloc = (dst % NLOC) // WIN
    e_lists = []
    for m in range(NCORES):
        per_w = []
        for w in range(NW):
            el = np.nonzero((core_of == m) & (wloc == w))[0]
            el = el[np.argsort(src[el], kind="stable")]   # HBM locality for gathers
            per_w.append(el)
        e_lists.append(per_w)
    Mw = [max(1, max((len(e_lists[m][w]) + 127) // 128 for m in range(NCORES)))
          for w in range(NW)]
    n_chunks = int(sum(Mw))

    src_row = np.zeros((NCORES, n_chunks * 128), np.int64)
    dst_loc = np.zeros((NCORES, n_chunks * 128), np.int64)
    oh_ev = np.zeros((NCORES, n_chunks, 128, 128), np.float32)
    for m in range(NCORES):
        ci = 0
        for w in range(NW):
            el = e_lists[m][w]
            ne = len(el)
            npad = Mw[w] * 128
            s_pad = np.zeros(npad, np.int64)
            d_pad = np.zeros(npad, np.int64)
            v_pad = np.zeros(npad, np.int64)
            valid = np.zeros(npad, bool)
            s_pad[:ne] = _row_of(src[el])
            d_pad[:ne] = dst[el] - m * NLOC
            v_pad[:ne] = dst[el] - m * NLOC - w * WIN
            valid[:ne] = True
            src_row[m, ci*128:(ci+Mw[w])*128] = s_pad
            dst_loc[m, ci*128:(ci+Mw[w])*128] = d_pad
            for k in range(Mw[w]):
                sl = slice(k * 128, (k + 1) * 128)
                vv, va = v_pad[sl], valid[sl]
                rows = np.nonzero(va)[0]
                oh_ev[m, ci + k, rows, vv[va]] = 1.0
            ci += Mw[w]

    f32 = np.float32
    attn1 = np.asarray(attn1, f32); attn2 = np.asarray(attn2, f32)
    W1s = np.asarray(W1s, f32); W1d = np.asarray(W1d, f32)
    W2s = np.asarray(W2s, f32); W2d = np.asarray(W2d, f32)
    W1r = np.asarray(W1r, f32); W2r = np.asarray(W2r, f32)
    b1s = np.asarray(b1s, f32); b1d = np.asarray(b1d, f32)
    b2s = np.asarray(b2s, f32); b2d = np.asarray(b2d, f32)
    b1r = np.asarray(b1r, f32); b2r = np.asarray(b2r, f32)
    Wp = np.asarray(Wp, f32); bp = np.asarray(bp, f32)

    T1s, b1s_r, T1d, b1d_r, W1r_p, b1r_p, inv1, rowperm1, nps1 = _fold_tables(
        W1s, b1s, W1d, b1d, attn1, W1r, b1r, D1, TW1)
    # conv2 consumes h1 in conv1-permuted basis: permute W2 rows first
    T2s, b2s_r, T2d, b2d_r, W2r_p, b2r_p, inv2, rowperm2, nps2 = _fold_tables(
        W2s[rowperm1], b2s, W2d[rowperm1], b2d, attn2, W2r[rowperm1], b2r, D2, TW2)

    W1W = 2 * TW1 + HID            # 1792
    W2W = 2 * TW2 + H * D2         # 3328
    W1cat = np.concatenate([T1s, T1d, W1r_p], axis=1)
    b1cat = np.zeros((128, W1W), f32)
    b1cat[0, 0:TW1] = b1s_r; b1cat[0, TW1:2*TW1] = b1d_r; b1cat[0, 2*TW1:] = b1r_p
    W2cat = np.concatenate([T2s, T2d, W2r_p], axis=1)
    b2cat = np.zeros((128, W2W), f32)
    b2cat[0, 0:TW2] = b2s_r; b2cat[0, TW2:2*TW2] = b2d_r; b2cat[0, 2*TW2:] = b2r_p

    # projection: out = sum_h relu_h_perm @ Wp[perm2_h]  (outer relu is identity
    # because both summands are >= 0)
    perm2 = [np.concatenate([np.nonzero(0.4*attn2[h] >= 0)[0],
                             np.nonzero(~(0.4*attn2[h] >= 0))[0]]) for h in range(H)]
    wpcat = np.concatenate([Wp[perm2[0]], Wp[perm2[1]]], axis=0)   # [1024, 512]

    bpcat = np.zeros((128, OUT), f32)
    bpcat[0, :] = bp
    has_bias = bool(max(float(np.abs(b).max()) for b in
                        (b1s, b1d, b1r, b2s, b2d, b2r, bp)) > 0)

    ident = np.eye(128, dtype=f32)
    ebias = np.zeros((128, 128), f32); ebias[0, :] = 1.0
    inv1_t = np.tile(inv1.reshape(1, -1), (128, 1))
    inv2_t = np.tile(inv2.reshape(1, -1), (128, 1))

    shared = {
        "w1cat": _bf16(W1cat), "b1cat": _bf16(b1cat),
        "w2cat": _bf16(W2cat), "b2cat": _bf16(b2cat),
        "wp": _bf16(wpcat), "bpcat": _bf16(bpcat),
        "ident": _bf16(ident), "ebias": _bf16(ebias),
        "inv1": _bf16(inv1_t), "inv2": _bf16(inv2_t),
    }
    in_maps = []
    for m in range(NCORES):
        xm = x[m*NLOC:(m+1)*NLOC]
        xT = np.zeros((IN, 1280), f32)
        xT[:, :NLOC] = xm.T
        im = dict(shared)
        im["xt"] = _bf16(xT)
        im["sidx"] = _pack_idx16(src_row[m])
        im["didx16"] = _pack_idx16(dst_loc[m])
        im["ohev"] = _bf16(oh_ev[m])
        in_maps.append(im)
    return in_maps, Mw, n_chunks, has_bias, nps1, nps2


def _blocks(mw):
    out, c = [], 0
    while c < mw:
        k = min(KBLK, mw - c)
        out.append((c, k))
        c += k
    return out


def _build_program(Mw, n_chunks, has_bias=False, repeat=1,
                   nps1=(128, 128), nps2=(256, 256), ablate=()):
    import sys
    if "/opt/trn_rl_repo" not in sys.path:
        sys.path.insert(0, "/opt/trn_rl_repo")
    import concourse.bass as bass
    import concourse.bacc as bacc
    import concourse.mybir as mybir
    import concourse.tile as tile

    dt = mybir.dt
    AF = mybir.ActivationFunctionType
    AL = mybir.AluOpType
    AX = mybir.AxisListType

    nc = bacc.Bacc("TRN2", target_bir_lowering=False, debug=False,
                   num_devices=NCORES)

    W1W = 2 * TW1 + HID
    W2W = 2 * TW2 + H * D2
    RG = [list(range(NCORES))]
    NOWN = NW * WIN          # 1280 (padded own-rows)
    PIECE_IN = WIN * AGW     # 256
    PIECE_OUT = NCORES * WIN * AGW   # 2048

    xt_d = nc.dram_tensor("xt", [IN, 1280], dt.bfloat16, kind="ExternalInput")
    w1_d = nc.dram_tensor("w1cat", [IN, W1W], dt.bfloat16, kind="ExternalInput")
    b1_d = nc.dram_tensor("b1cat", [128, W1W], dt.bfloat16, kind="ExternalInput")
    w2_d = nc.dram_tensor("w2cat", [HID, W2W], dt.bfloat16, kind="ExternalInput")
    b2_d = nc.dram_tensor("b2cat", [128, W2W], dt.bfloat16, kind="ExternalInput")
    wp_d = nc.dram_tensor("wp", [2 * D2, OUT], dt.bfloat16, kind="ExternalInput")
    bp_d = nc.dram_tensor("bpcat", [128, OUT], dt.bfloat16, kind="ExternalInput")
    id_d = nc.dram_tensor("ident", [128, 128], dt.bfloat16, kind="ExternalInput")
    eb_d = nc.dram_tensor("ebias", [128, 128], dt.bfloat16, kind="ExternalInput")
    iv1_d = nc.dram_tensor("inv1", [128, HID], dt.bfloat16, kind="ExternalInput")
    iv2_d = nc.dram_tensor("inv2", [128, H * D2], dt.bfloat16, kind="ExternalInput")
    si_d = nc.dram_tensor("sidx", [128, n_chunks * 8], dt.int16, kind="ExternalInput")
    di16_d = nc.dram_tensor("didx16", [128, n_chunks * 8], dt.int16,
                            kind="ExternalInput")
    ohev_d = nc.dram_tensor("ohev", [n_chunks, 128, 128], dt.bfloat16,
                            kind="ExternalInput")

    t1_own = nc.dram_tensor("t1_own", [NOWN, TW1], dt.bfloat16, kind="Internal")
    t1_full = nc.dram_tensor("t1_full", [N2, TW1], dt.bfloat16, kind="Internal",
                             addr_space="Shared")
    fd1_dram = nc.dram_tensor("fd1_dram", [NOWN, TW1], dt.bfloat16, kind="Internal")
    t2_own = nc.dram_tensor("t2_own", [NOWN, TW2], dt.bfloat16, kind="Internal")
    t2_full = nc.dram_tensor("t2_full", [N2, TW2], dt.bfloat16, kind="Internal",
                             addr_space="Shared")
    fd2_dram = nc.dram_tensor("fd2_dram", [NOWN, TW2], dt.bfloat16, kind="Internal")
    out_d = nc.dram_tensor("out", [NLOC, OUT], dt.float32, kind="ExternalOutput")

    seg1 = [(0, nps1[0]), (nps1[0], D1), (D1, D1 + nps1[1]), (D1 + nps1[1], 2 * D1)]
    seg2 = [(0, nps2[0]), (nps2[0], D2), (D2, D2 + nps2[1]), (D2 + nps2[1], 2 * D2)]

    def mm_cols(ncols):
        splits, c = [], 0
        while c < ncols:
            n_ = min(512, ncols - c)
            splits.append((c, c + n_))
            c += n_
        return splits

    with tile.TileContext(nc) as tc:
        with tc.tile_pool(name="cst", bufs=1) as cst, \
             tc.tile_pool(name="res", bufs=1) as resid:

            ident = cst.tile([128, 128], dt.bfloat16)
            nc.sync.dma_start(out=ident[:, :], in_=id_d[:, :])
            ebias = cst.tile([128, 128], dt.bfloat16)
            nc.sync.dma_start(out=ebias[:, :], in_=eb_d[:, :])
            inv1 = cst.tile([128, HID], dt.bfloat16)
            nc.sync.dma_start(out=inv1[:, :], in_=iv1_d[:, :])
            inv2 = cst.tile([128, H * D2], dt.bfloat16)
            nc.sync.dma_start(out=inv2[:, :], in_=iv2_d[:, :])
            sidx = cst.tile([128, n_chunks * 8], dt.int16)
            nc.sync.dma_start(out=sidx[:, :], in_=si_d[:, :])
            didx16 = cst.tile([128, n_chunks * 8], dt.int16)
            nc.sync.dma_start(out=didx16[:, :], in_=di16_d[:, :])

            rs1 = resid.tile([128, NW * HID], dt.bfloat16)
            h1 = resid.tile([128, NW * HID], dt.bfloat16)
            rs2 = resid.tile([128, NW * H * D2], dt.bfloat16)
            fd1w = resid.tile([128, NW * HID], dt.bfloat16)
            fd2w = resid.tile([128, NW * H * D2], dt.bfloat16)

            cb1 = [0]
            for w in range(NW):
                cb1.append(cb1[-1] + Mw[w])

            for rep in range(repeat):
                # ============ conv1 node matmuls + chunked AG1 ============
                with tc.tile_pool(name="n1", bufs=2) as n1p, \
                     tc.tile_pool(name="n1w", bufs=1) as n1w, \
                     tc.tile_pool(name="n1ps", bufs=2, space="PSUM") as n1ps:
                    xt_sb = n1w.tile([128, 8 * 1280], dt.bfloat16)
                    for k in range(8):
                        nc.sync.dma_start(out=xt_sb[:, k*1280:(k+1)*1280],
                                          in_=xt_d[k*128:(k+1)*128, :])
                    w1 = n1w.tile([128, 8 * W1W], dt.bfloat16)
                    for k in range(8):
                        nc.sync.dma_start(out=w1[:, k*W1W:(k+1)*W1W],
                                          in_=w1_d[k*128:(k+1)*128, :])
                    b1 = n1w.tile([128, W1W], dt.bfloat16)
                    nc.sync.dma_start(out=b1[:, :], in_=b1_d[:, :])
                    for nt in range(NW):
                        ps = n1ps.tile([128, W1W], dt.float32, space="PSUM")
                        for k in range(8):
                            for (c0, c1) in mm_cols(W1W):
                                nc.tensor.matmul(
                                    ps[:, c0:c1],
                                    lhsT=xt_sb[:, k*1280 + nt*128: k*1280 + (nt+1)*128],
                                    rhs=w1[:, k*W1W+c0:k*W1W+c1],
                                    start=(k == 0), stop=(k == 7 and not has_bias))
                        if has_bias:
                            for (c0, c1) in mm_cols(W1W):
                                nc.tensor.matmul(ps[:, c0:c1], lhsT=ebias[:, :],
                                                 rhs=b1[:, c0:c1], start=False,
                                                 stop=True)
                        t1t = n1p.tile([128, TW1], dt.bfloat16, tag="t1t")
                        nc.vector.tensor_copy(out=t1t[:, :], in_=ps[:, 0:TW1])
                        fdt = n1p.tile([128, TW1], dt.bfloat16, tag="fdt")
                        nc.vector.tensor_copy(out=fdt[:, :], in_=ps[:, TW1:2*TW1])
                        nc.vector.tensor_copy(out=fd1w[:, nt*HID:(nt+1)*HID],
                                              in_=ps[:, TW1:TW1+HID])
                        nc.vector.tensor_copy(out=rs1[:, nt*HID:(nt+1)*HID],
                                              in_=ps[:, 2*TW1:W1W])
                        nc.sync.dma_start(out=t1_own[nt*128:(nt+1)*128, :],
                                          in_=t1t[:, :])
                        nc.sync.dma_start(out=fd1_dram[nt*128:(nt+1)*128, :],
                                          in_=fdt[:, :])
                        if "ag" not in ablate and nt % AGW == AGW - 1:
                            p = nt // AGW
                            nc.gpsimd.collective_compute(
                                "AllGather", mybir.AluOpType.bypass,
                                replica_groups=RG,
                                ins=[t1_own[p*PIECE_IN:(p+1)*PIECE_IN, :]],
                                outs=[t1_full[p*PIECE_OUT:(p+1)*PIECE_OUT, :]])

                # ======= pipelined: conv1 edge(w) -> conv2 node(w) -> AG2 ======
                with tc.tile_pool(name="e1a", bufs=3) as e1a, \
                     tc.tile_pool(name="e1g", bufs=2) as e1g, \
                     tc.tile_pool(name="e1b", bufs=2) as e1b, \
                     tc.tile_pool(name="n2", bufs=2) as n2p, \
                     tc.tile_pool(name="n2w", bufs=1) as n2w, \
                     tc.tile_pool(name="e1agg", bufs=2, space="PSUM") as e1agg, \
                     tc.tile_pool(name="n2ps", bufs=1, space="PSUM") as n2ps:
                    w2 = n2w.tile([128, 4 * W2W], dt.bfloat16)
                    for k in range(4):
                        nc.sync.dma_start(out=w2[:, k*W2W:(k+1)*W2W],
                                          in_=w2_d[k*128:(k+1)*128, :])
                    b2 = n2w.tile([128, W2W], dt.bfloat16)
                    nc.sync.dma_start(out=b2[:, :], in_=b2_d[:, :])
                    for w in range(NW):
                        mw = Mw[w]
                        cbase = cb1[w]
                        # ---------------- conv1 edge window w ----------------
                        agg = e1agg.tile([128, HID], dt.float32, space="PSUM",
                                         tag="agg")
                        den = e1agg.tile([128, H], dt.float32, space="PSUM",
                                         tag="den")
                        for bi, (c0, kb) in enumerate(_blocks(mw)):
                            ci = cbase + c0
                            first = (bi == 0)
                            last = (c0 + kb == mw)
                            Z = e1a.tile([128, KBLK * TW1], dt.bfloat16, tag="Z")
                            Zv = Z[:, 0:kb*TW1].rearrange("p (k t) -> p k t", t=TW1)
                            if "gather" not in ablate:
                                nc.gpsimd.dma_gather(
                                    out_ap=Zv, in_ap=t1_full[:, :],
                                    idxs_ap=sidx[:, ci*8:(ci+kb)*8],
                                    num_idxs=kb*128, num_idxs_reg=kb*128,
                                    elem_size=TW1)
                                gfd = e1g.tile([128, KBLK * TW1], dt.bfloat16,
                                               tag="gfd")
                                nc.gpsimd.dma_gather(
                                    out_ap=gfd[:, 0:kb*TW1].rearrange(
                                        "p (k t) -> p k t", t=TW1),
                                    in_ap=fd1_dram[:, :],
                                    idxs_ap=didx16[:, ci*8:(ci+kb)*8],
                                    num_idxs=kb*128, num_idxs_reg=kb*128,
                                    elem_size=TW1)
                                nc.vector.tensor_tensor(
                                    out=Z[:, 0:kb*TW1], in0=Z[:, 0:kb*TW1],
                                    in1=gfd[:, 0:kb*TW1], op=AL.add)
                            oh = e1a.tile([128, KBLK * 128], dt.bfloat16, tag="oh")
                            nc.sync.dma_start(
                                out=oh[:, 0:kb*128].rearrange("p (c o) -> p c o",
                                                              o=128),
                                in_=ohev_d[ci:ci+kb, :, :].rearrange("c p o -> p c o"))
                            rp = e1b.tile([128, 2 * KBLK], dt.float32, tag="rp")
                            rn = e1b.tile([128, 2 * KBLK], dt.float32, tag="rn")
                            scr = e1b.tile([128, 256], dt.bfloat16, tag="scr")
                            # conv1 abs-reduces: h0 on ACT (accum_out), h1 on DVE
                            for c in range(kb):
                                s0, s1 = seg1[0]
                                nc.scalar.activation(
                                    out=scr[:, 0:s1-s0], in_=Zv[:, c, s0:s1],
                                    func=AF.Abs, accum_out=rp[:, 2*c:2*c+1])
                                s0, s1 = seg1[1]
                                nc.scalar.activation(
                                    out=scr[:, 0:s1-s0], in_=Zv[:, c, s0:s1],
                                    func=AF.Abs, accum_out=rn[:, 2*c:2*c+1])
                            s0, s1 = seg1[2]
                            nc.vector.tensor_reduce(
                                out=rp[:, 1:2*kb:2], in_=Zv[:, :, s0:s1],
                                axis=AX.X, op=AL.add, apply_absolute_value=True)
                            s0, s1 = seg1[3]
                            nc.vector.tensor_reduce(
                                out=rn[:, 1:2*kb:2], in_=Zv[:, :, s0:s1],
                                axis=AX.X, op=AL.add, apply_absolute_value=True)
                            et = e1b.tile([128, 2 * KBLK], dt.float32, tag="et")
                            nc.vector.tensor_tensor(out=et[:, 0:2*kb],
                                                    in0=rp[:, 0:2*kb],
                                                    in1=rn[:, 0:2*kb], op=AL.subtract)
                            nc.vector.tensor_tensor(
                                out=et[:, 0:2*kb].rearrange("p (k h) -> p k h", h=2),
                                in0=et[:, 0:2*kb].rearrange("p (k h) -> p k h", h=2),
                                in1=Zv[:, :, 2*D1:2*D1+2], op=AL.add)
                            ex = e1b.tile([128, 2 * KBLK], dt.bfloat16, tag="ex")
                            nc.scalar.activation(out=ex[:, 0:2*kb], in_=et[:, 0:2*kb],
                                                 func=AF.Exp)
                            zsv = Zv[:, :, 0:2*D1].rearrange(
                                "p k (h d) -> p k h d", d=D1)
                            nc.vector.tensor_tensor(
                                out=zsv, in0=zsv,
                                in1=ex[:, 0:2*kb].rearrange(
                                    "p (k h u) -> p k h u", h=2, u=1)
                                    .broadcast_to([128, kb, 2, D1]),
                                op=AL.mult)
                            for c in range(kb):
                                st = first and (c == 0)
                                sp = last and (c == kb - 1)
                                nc.tensor.matmul(
                                    agg[:, :], lhsT=oh[:, c*128:(c+1)*128],
                                    rhs=Z[:, c*TW1:c*TW1+HID], start=st, stop=sp)
                                nc.tensor.matmul(
                                    den[:, :], lhsT=oh[:, c*128:(c+1)*128],
                                    rhs=ex[:, 2*c:2*c+2], start=st, stop=sp)
                        # ---- conv1 window epilogue ----
                        dsb = e1b.tile([128, H], dt.float32, tag="dsb")
                        nc.vector.tensor_scalar_add(dsb[:, :], den[:, :], 1e-20)
                        rcp = e1b.tile([128, H], dt.float32, tag="rcp")
                        nc.vector.reciprocal(rcp[:, :], dsb[:, :])
                        hw = h1[:, w*HID:(w+1)*HID]
                        nc.vector.tensor_scalar_mul(hw[:, 0:D1], agg[:, 0:D1],
                                                    rcp[:, 0:1])
                        nc.vector.tensor_scalar_mul(hw[:, D1:HID], agg[:, D1:HID],
                                                    rcp[:, 1:2])
                        nc.vector.tensor_tensor(out=hw[:, :], in0=hw[:, :],
                                                in1=fd1w[:, w*HID:(w+1)*HID],
                                                op=AL.subtract)
                        nc.vector.tensor_tensor(out=hw[:, :], in0=hw[:, :],
                                                in1=inv1[:, :], op=AL.mult)
                        nc.vector.tensor_tensor(out=hw[:, :], in0=hw[:, :],
                                                in1=rs1[:, w*HID:(w+1)*HID],
                                                op=AL.add)
                        nc.vector.tensor_scalar_max(hw[:, :], hw[:, :], 0.0)

                        # ---------------- conv2 node matmuls window w --------
                        tp = n2ps.tile([128, 128], dt.bfloat16, space="PSUM",
                                       tag="tp")
                        hT = n2p.tile([128, 512], dt.bfloat16, tag="hT")
                        for j in range(4):
                            nc.tensor.transpose(
                                out=tp[:, :],
                                in_=h1[:, w*HID+j*128:w*HID+(j+1)*128],
                                identity=ident[:, :])
                            nc.vector.tensor_copy(out=hT[:, j*128:(j+1)*128],
                                                  in_=tp[:, :])
                        t2t = n2p.tile([128, TW2], dt.bfloat16, tag="t2t")
                        fdt = n2p.tile([128, TW2], dt.bfloat16, tag="fdt2")
                        ps = n2ps.tile([128, TW2], dt.float32, space="PSUM",
                                       tag="ps2")
                        for (base, hw_, kind) in ((0, TW2, "t2"), (TW2, TW2, "fd"),
                                                  (2*TW2, H*D2, "rs")):
                            spans = mm_cols(hw_)
                            for k in range(4):
                                for (c0, c1) in spans:
                                    nc.tensor.matmul(
                                        ps[:, c0:c1], lhsT=hT[:, k*128:(k+1)*128],
                                        rhs=w2[:, k*W2W+base+c0:k*W2W+base+c1],
                                        start=(k == 0),
                                        stop=(k == 3 and not has_bias))
                            if has_bias:
                                for (c0, c1) in spans:
                                    nc.tensor.matmul(ps[:, c0:c1], lhsT=ebias[:, :],
                                                     rhs=b2[:, base+c0:base+c1],
                                                     start=False, stop=True)
                            if kind == "t2":
                                nc.vector.tensor_copy(out=t2t[:, :], in_=ps[:, 0:TW2])
                            elif kind == "fd":
                                nc.vector.tensor_copy(out=fdt[:, :], in_=ps[:, 0:TW2])
                                nc.vector.tensor_copy(
                                    out=fd2w[:, w*1024:(w+1)*1024], in_=ps[:, 0:1024])
                            else:
                                nc.vector.tensor_copy(
                                    out=rs2[:, w*1024:(w+1)*1024], in_=ps[:, 0:1024])
                        nc.sync.dma_start(out=t2_own[w*128:(w+1)*128, :],
                                          in_=t2t[:, :])
                        nc.sync.dma_start(out=fd2_dram[w*128:(w+1)*128, :],
                                          in_=fdt[:, :])
                        if "ag" not in ablate and w % AGW == AGW - 1:
                            p = w // AGW
                            nc.gpsimd.collective_compute(
                                "AllGather", mybir.AluOpType.bypass,
                                replica_groups=RG,
                                ins=[t2_own[p*PIECE_IN:(p+1)*PIECE_IN, :]],
                                outs=[t2_full[p*PIECE_OUT:(p+1)*PIECE_OUT, :]])

                # ============== conv2 edge phase + projection ==============
                with tc.tile_pool(name="e2a", bufs=3) as e2a, \
                     tc.tile_pool(name="e2g", bufs=2) as e2g, \
                     tc.tile_pool(name="e2b", bufs=2) as e2b, \
                     tc.tile_pool(name="e2w", bufs=1) as e2w, \
                     tc.tile_pool(name="e2agg", bufs=2, space="PSUM") as e2agg, \
                     tc.tile_pool(name="e2pf", bufs=2, space="PSUM") as e2pf:
                    wp = e2w.tile([128, 8 * OUT], dt.bfloat16)
                    for k in range(8):
                        nc.sync.dma_start(out=wp[:, k*OUT:(k+1)*OUT],
                                          in_=wp_d[k*128:(k+1)*128, :])
                    bpt = e2w.tile([128, OUT], dt.bfloat16)
                    nc.sync.dma_start(out=bpt[:, :], in_=bp_d[:, :])
                    for w in range(NW):
                        mw = Mw[w]
                        cbase = cb1[w]
                        agg0 = e2agg.tile([128, D2], dt.float32, space="PSUM",
                                          tag="agg0")
                        agg1 = e2agg.tile([128, D2], dt.float32, space="PSUM",
                                          tag="agg1")
                        den = e2agg.tile([128, H], dt.float32, space="PSUM",
                                         tag="den")
                        for bi, (c0, kb) in enumerate(_blocks(mw)):
                            ci = cbase + c0
                            first = (bi == 0)
                            last = (c0 + kb == mw)
                            Z = e2a.tile([128, KBLK * TW2], dt.bfloat16, tag="Z")
                            Zv = Z[:, 0:kb*TW2].rearrange("p (k t) -> p k t", t=TW2)
                            if "gather" not in ablate:
                                nc.gpsimd.dma_gather(
                                    out_ap=Zv, in_ap=t2_full[:, :],
                                    idxs_ap=sidx[:, ci*8:(ci+kb)*8],
                                    num_idxs=kb*128, num_idxs_reg=kb*128,
                                    elem_size=TW2)
                                gfd = e2g.tile([128, KBLK * TW2], dt.bfloat16,
                                               tag="gfd")
                                nc.gpsimd.dma_gather(
                                    out_ap=gfd[:, 0:kb*TW2].rearrange(
                                        "p (k t) -> p k t", t=TW2),
                                    in_ap=fd2_dram[:, :],
                                    idxs_ap=didx16[:, ci*8:(ci+kb)*8],
                                    num_idxs=kb*128, num_idxs_reg=kb*128,
                                    elem_size=TW2)
                                nc.vector.tensor_tensor(
                                    out=Z[:, 0:kb*TW2], in0=Z[:, 0:kb*TW2],
                                    in1=gfd[:, 0:kb*TW2], op=AL.add)
                            oh = e2a.tile([128, KBLK * 128], dt.bfloat16, tag="oh")
                            nc.sync.dma_start(
                                out=oh[:, 0:kb*128].rearrange("p (c o) -> p c o",
                                                              o=128),
                                in_=ohev_d[ci:ci+kb, :, :].rearrange("c p o -> p c o"))
                            rp = e2b.tile([128, 2 * KBLK], dt.float32, tag="rp")
                            rn = e2b.tile([128, 2 * KBLK], dt.float32, tag="rn")
                            scr2 = e2b.tile([128, 512], dt.bfloat16, tag="scr2")
                            # conv2 abs-reduces: h0 on DVE (batched), h1 on ACT
                            s0, s1 = seg2[0]
                            nc.vector.tensor_reduce(
                                out=rp[:, 0:2*kb:2], in_=Zv[:, :, s0:s1],
                                axis=AX.X, op=AL.add, apply_absolute_value=True)
                            s0, s1 = seg2[1]
                            nc.vector.tensor_reduce(
                                out=rn[:, 0:2*kb:2], in_=Zv[:, :, s0:s1],
                                axis=AX.X, op=AL.add, apply_absolute_value=True)
                            for c in range(kb):
                                s0, s1 = seg2[2]
                                nc.scalar.activation(
                                    out=scr2[:, 0:s1-s0], in_=Zv[:, c, s0:s1],
                                    func=AF.Abs, accum_out=rp[:, 2*c+1:2*c+2])
                                s0, s1 = seg2[3]
                                nc.scalar.activation(
                                    out=scr2[:, 0:s1-s0], in_=Zv[:, c, s0:s1],
                                    func=AF.Abs, accum_out=rn[:, 2*c+1:2*c+2])
                            et = e2b.tile([128, 2 * KBLK], dt.float32, tag="et")
                            nc.vector.tensor_tensor(out=et[:, 0:2*kb],
                                                    in0=rp[:, 0:2*kb],
                                                    in1=rn[:, 0:2*kb], op=AL.subtract)
                            nc.vector.tensor_tensor(
                                out=et[:, 0:2*kb].rearrange("p (k h) -> p k h", h=2),
                                in0=et[:, 0:2*kb].rearrange("p (k h) -> p k h", h=2),
                                in1=Zv[:, :, 2*D2:2*D2+2], op=AL.add)
                            ex = e2b.tile([128, 2 * KBLK], dt.bfloat16, tag="ex")
                            nc.scalar.activation(out=ex[:, 0:2*kb], in_=et[:, 0:2*kb],
                                                 func=AF.Exp)
                            zsv = Zv[:, :, 0:2*D2].rearrange(
                                "p k (h d) -> p k h d", d=D2)
                            nc.vector.tensor_tensor(
                                out=zsv, in0=zsv,
                                in1=ex[:, 0:2*kb].rearrange(
                                    "p (k h u) -> p k h u", h=2, u=1)
                                    .broadcast_to([128, kb, 2, D2]),
                                op=AL.mult)
                            for c in range(kb):
                                st = first and (c == 0)
                                sp = last and (c == kb - 1)
                                nc.tensor.matmul(
                                    agg0[:, :], lhsT=oh[:, c*128:(c+1)*128],
                                    rhs=Z[:, c*TW2:c*TW2+D2], start=st, stop=sp)
                                nc.tensor.matmul(
                                    agg1[:, :], lhsT=oh[:, c*128:(c+1)*128],
                                    rhs=Z[:, c*TW2+D2:c*TW2+2*D2], start=st, stop=sp)
                                nc.tensor.matmul(
                                    den[:, :], lhsT=oh[:, c*128:(c+1)*128],
                                    rhs=ex[:, 2*c:2*c+2], start=st, stop=sp)
                        # ---- epilogue: r_h = relu((agg_h/den_h - fd_h)*inv + rs) --
                        dsb = e2b.tile([128, H], dt.float32, tag="dsb")
                        nc.vector.tensor_scalar_add(dsb[:, :], den[:, :], 1e-20)
                        rcp = e2b.tile([128, H], dt.float32, tag="rcp")
                        nc.vector.reciprocal(rcp[:, :], dsb[:, :])
                        rh = e2b.tile([128, 2 * D2], dt.bfloat16, tag="rh")
                        nc.vector.tensor_scalar_mul(rh[:, 0:D2], agg0[:, :],
                                                    rcp[:, 0:1])
                        nc.vector.tensor_scalar_mul(rh[:, D2:2*D2], agg1[:, :],
                                                    rcp[:, 1:2])
                        nc.vector.tensor_tensor(out=rh[:, :], in0=rh[:, :],
                                                in1=fd2w[:, w*1024:(w+1)*1024],
                                                op=AL.subtract)
                        nc.vector.tensor_tensor(out=rh[:, :], in0=rh[:, :],
                                                in1=inv2[:, :], op=AL.mult)
                        nc.vector.tensor_tensor(out=rh[:, :], in0=rh[:, :],
                                                in1=rs2[:, w*1024:(w+1)*1024],
                                                op=AL.add)
                        nc.vector.tensor_scalar_max(rh[:, :], rh[:, :], 0.0)
                        # ---- projection: out = sum_h r_h @ wp_h ----
                        pf = e2pf.tile([128, OUT], dt.float32, space="PSUM", tag="pf")
                        rT = e2b.tile([128, 8 * 128], dt.bfloat16, tag="rT")
                        for j in range(8):
                            tpv = pf[:, j*64:(j+1)*64].bitcast(dt.bfloat16)
                            nc.tensor.transpose(out=tpv, in_=rh[:, j*128:(j+1)*128],
                                                identity=ident[:, :])
                            nc.vector.tensor_copy(out=rT[:, j*128:(j+1)*128],
                                                  in_=tpv)
                        for k in range(8):
                            nc.tensor.matmul(pf[:, :], lhsT=rT[:, k*128:(k+1)*128],
                                             rhs=wp[:, k*OUT:(k+1)*OUT],
                                             start=(k == 0),
                                             stop=(k == 7 and not has_bias))
                        if has_bias:
                            nc.tensor.matmul(pf[:, :], lhsT=ebias[:, :],
                                             rhs=bpt[:, :], start=False, stop=True)
                        of = e2b.tile([128, OUT], dt.float32, tag="of")
                        nc.vector.tensor_copy(out=of[:, :], in_=pf[:, :])
                        rows = min(128, NLOC - w * 128)
                        nc.sync.dma_start(out=out_d[w*128:w*128+rows, :],
                                          in_=of[:rows, :])

    nc.compile()
    return nc


def kernel(**inputs) -> np.ndarray:
    import sys
    if "/opt/trn_rl_repo" not in sys.path:
        sys.path.insert(0, "/opt/trn_rl_repo")
    from concourse.bass_utils import run_bass_kernel_spmd

    in_maps, Mw, n_chunks, has_bias, nps1, nps2 = _host_prep(**inputs)
    key = ("prog", tuple(Mw), has_bias, nps1, nps2)
    if key not in _CACHE:
        _CACHE[key] = _build_program(Mw, n_chunks, has_bias, repeat=1,
                                     nps1=nps1, nps2=nps2)
    nc = _CACHE[key]
    res = run_bass_kernel_spmd(nc, in_maps, core_ids=list(range(NCORES)))
    return np.concatenate([res.results[m]["out"] for m in range(NCORES)], axis=0)
